# revision 56
# baseline (speedup 1.0000x reference)
"""Multi-head attention block for Trainium2, 8-core data-parallel SPMD.

Computes, per batch element b (one NeuronCore each):
    qkv = x @ w_qkv ; q,k,v split into 16 heads of dim 64
    attn = softmax(q @ k^T / sqrt(64)) ; out = (attn @ v) @ w_out + b_out

Design notes (v2, bf16, gapless PE):
  - The kernel is paced by the PE stream; any PE idle gap costs ~1.8us of
    p-state ramp (1.2GHz for 3us after a restart), so every phase is
    arranged to keep the PE dense:
      * warmup filler matmuls ramp the clock while the first x chunks DMA in
      * transposes of x are fused into the v-projection loop, chasing the
        per-chunk x DMAs (sync queue) while weights stream on the scalar
        queue (host pre-blocks w_qkv so every weight DMA is contiguous)
      * attention uses a 3-deep PSUM rotation for the S tiles (6 banks) so
        the S matmuls never wait on the exp (ACT) stream; AV runs two
        passes (one 512-half at a time) in a single accumulator bank, and
        the q/k projection interleave uses the last bank with copy-paced
        group boundaries
      * the AV stream trails the S stream by about one head; after the last
        S slot the remaining AV work interleaves with the out-projection
        openings (ci=0..6), and the ci=7 close of each row chunk is
        deferred until the final head's normalize has landed
  - All PE operands are bf16 (cast on host); PSUM accumulation stays fp32.
  - Softmax: one [128,1024] exp per (head, k-chunk) on ACT (scale folded
    in); row-sums ride along as a ones-column in the augmented-v matmuls;
    normalize = reciprocal (DVE) -> partition-broadcast (GpSimd) -> mult.
  - PSUM budget (8 banks): sT [P,1024]x3 "mm" (6) + proj [P,512] (1) +
    acc [P,512] (1).  The "mm" tag is reused by transposes / v-proj /
    out-proj in the other phases.
"""

import sys
from collections import deque

if "/opt/trn_rl_repo" not in sys.path:
    sys.path.insert(0, "/opt/trn_rl_repo")

import numpy as np

B = 8
N = 1024  # sequence length
C = 1024  # model dim
H = 16  # heads
D = 64  # head dim
P = 128  # partitions
NT = N // P  # seq chunks
CT = C // P  # channel chunks
HP = H // 2  # head pairs
SCALE = D ** -0.5
HF = C // 512  # 512-wide halves per 1024 row
TRAIL = 18  # av entries kept in flight (~one head incl. stages)
N_WARM = 3  # warmup filler matmuls

_CACHE = {}


def _build_program():
    from concourse import bacc, mybir
    import concourse.tile as tile
    from concourse.masks import make_identity

    f32 = mybir.dt.float32
    bf16 = mybir.dt.bfloat16
    Exp = mybir.ActivationFunctionType.Exp

    nc = bacc.Bacc("TRN2", target_bir_lowering=False, debug=False)
    x_d = nc.declare_dram_parameter("x", [N, C], bf16, isOutput=False)
    wv_d = nc.declare_dram_parameter("w_v", [C, C], bf16, isOutput=False)
    # per (pair, q/k): [128, 1024] block with [p, ci*128+c] = w[ci*128+p, base+c]
    wqk_d = nc.declare_dram_parameter("w_qk", [2 * HP * P, C], bf16, isOutput=False)
    wout_d = nc.declare_dram_parameter("w_out", [C, C], bf16, isOutput=False)
    bout_d = nc.declare_dram_parameter("b_out", [1, C], bf16, isOutput=False)
    out_d = nc.declare_dram_parameter("out", [N, C], f32, isOutput=True)

    with tile.TileContext(nc) as tc:
        with (
            tc.tile_pool(name="consts", bufs=1) as consts,
            tc.tile_pool(name="xTo", bufs=1) as xT_pool,
            tc.tile_pool(name="vaug", bufs=NT) as vaug_pool,
            tc.tile_pool(name="psum", bufs=1, space="PSUM") as psum,
            tc.tile_pool(name="oTp", bufs=CT) as oT_pool,
            tc.tile_pool(name="io", bufs=6) as io_pool,
            tc.tile_pool(name="xin", bufs=NT) as xin_pool,
            tc.tile_pool(name="wv", bufs=CT) as wv_pool,
            tc.tile_pool(name="wo", bufs=CT) as wo_pool,
            tc.tile_pool(name="wqk", bufs=4) as wqk_pool,
            tc.tile_pool(name="qkT", bufs=2) as qkT_pool,
            tc.tile_pool(name="pT", bufs=14) as pT_pool,
            tc.tile_pool(name="oTu", bufs=2) as oTu_pool,
            tc.tile_pool(name="rsum", bufs=2) as rs_pool,
            tc.tile_pool(name="recip", bufs=4) as rc_pool,
            tc.tile_pool(name="bcs", bufs=4) as bcs_pool,
        ):
            # ---- consts (DVE/gpsimd, ordered so the PE can start asap) ----
            with tc.high_priority():
                ones_bf = consts.tile([P, 512], bf16, name="ones_bf", tag="ones_bf")
                nc.vector.memset(ones_bf, 1.0)
                identity_f32 = consts.tile(
                    [P, P], f32, name="identity_f32", tag="identity_f32"
                )
                make_identity(nc, identity_f32)
                identity = consts.tile([P, P], bf16, name="identity", tag="identity")
                nc.vector.tensor_copy(identity[:, :], identity_f32[:, :])

            # ---- input DMAs: x chunks on sync, weights on scalar ----
            xins = []
            for si in range(NT):
                xin = xin_pool.tile([P, N], bf16, name=f"xin{si}", tag="xin")
                # two half-column transfers: completion semaphores lag the
                # data by several transfers' worth of aggregation, so finer
                # transfers release the transpose chase sooner
                for hf in range(HF):
                    sl = slice(hf * 512, hf * 512 + 512)
                    nc.sync.dma_start(
                        out=xin[:, sl], in_=x_d[si * P : (si + 1) * P, sl]
                    )
                xins.append(xin)
            wv = []
            for ci in range(CT):
                w = wv_pool.tile([P, N], bf16, name=f"wv{ci}", tag="wv")
                # gpsimd is a third DMA initiator, idle early: the whole wv
                # stream goes there so scalar only carries wq0/wk0/bias/warm
                nc.gpsimd.dma_start(out=w[:, :], in_=wv_d[ci * P : (ci + 1) * P, :])
                wv.append(w)

            def load_wqk(t, eng):
                tiles = []
                for wi, nmc in ((0, "q"), (1, "k")):
                    w = wqk_pool.tile([P, C], bf16, name=f"w{nmc}{t}", tag="wqk")
                    r = (2 * t + wi) * P
                    eng.dma_start(out=w[:, :], in_=wqk_d[r : r + P, :])
                    tiles.append(w)
                return tiles

            wqk0 = load_wqk(0, nc.scalar)
            # dummy exp pulls the ACT_TABLE_LOAD into the prologue
            warm = consts.tile([1, 16], f32, name="act_warm", tag="act_warm")
            nc.scalar.activation(
                out=warm[0:1, :], in_=identity_f32[0:1, 0:16], func=Exp
            )

            # zero-padded k tiles: head j's k lives in rows [64j, 64j+64),
            # the other 64 rows stay zero forever.  A full [128,128]
            # stationary avoids the PE tile_size reconfiguration (~100ns
            # drain each way) that a 64-row stationary costs on every S pair.
            # Memsets on gpsimd (queued behind the wv loads) keep the early
            # DVE queue clear for the xTall copies.
            kz = [
                [
                    consts.tile([P, N], bf16, name=f"kz{par}{j}", tag=f"kz{par}{j}")
                    for j in range(2)
                ]
                for par in range(2)
            ]
            for par in range(2):
                nc.gpsimd.memset(kz[par][0][D:P, :], 0.0)
                nc.gpsimd.memset(kz[par][1][0:D, :], 0.0)

            # ---- SBUF working tiles ----
            xTall = xT_pool.tile([P, CT * N], bf16, name="xTall", tag="xTo")

            def xT(ci):
                return xTall[:, ci * N : (ci + 1) * N]

            vaug = [
                vaug_pool.tile([P, H * (D + 1)], bf16, name=f"vaug{i}", tag="vaug")
                for i in range(NT)
            ]
            oT = [
                oT_pool.tile([P, N], bf16, name=f"oT{i}", tag="oTp") for i in range(CT)
            ]

            def mm_tile(name, dtype=f32):
                return psum.tile([P, N], dtype, name=name, tag="mm", bufs=3)

            # ---- PE warmup fillers (ramp the clock during DMA-in) ----
            for i in range(N_WARM):
                fl = psum.tile([P, 512], f32, name=f"warm{i}", tag="acc", bufs=1)
                nc.tensor.matmul(
                    fl[:, :], ones_bf[:, 0:P], ones_bf[:, :], start=True, stop=True
                )

            # ---- transposes of x, fused into the v loop ----
            # tr tiles use the single-bank acc/proj slots (alternating), NOT
            # the 3-deep "mm" rotation: v accumulators stay open across many
            # interleaved ops there, and a tr tile landing on an open v slot
            # would deadlock the in-order PE queue.
            def emit_tr_group(si, split_copy=False):
                xin = xins[si]
                tag = "acc" if si % 2 == 0 else "proj"
                tr_ps = psum.tile([P, N], bf16, name=f"tr{si}", tag=tag, bufs=1)
                ops = []
                if si <= 4:
                    # dependency-free filler transposes at the group head keep
                    # the PE (and its p-state ramp) fed across the DMA
                    # semaphore lag of the xin chunk this group waits on
                    for _ in range(2):
                        ops.append(
                            lambda: nc.tensor.transpose(
                                tr_ps[:, 0:P], ones_bf[:, 0:P], identity
                            )
                        )
                for ci in range(CT):
                    ops.append(
                        lambda ci=ci: nc.tensor.transpose(
                            tr_ps[:, ci * P : (ci + 1) * P],
                            xin[:, ci * P : (ci + 1) * P],
                            identity,
                        )
                    )

                src = tr_ps.rearrange("p (ci c) -> p ci c", c=P)
                dst = xTall.rearrange("p (ci n) -> p ci n", n=N)

                def copy(lo, hi):
                    nc.vector.tensor_copy(
                        dst[:, lo:hi, si * P : (si + 1) * P], src[:, lo:hi, :]
                    )

                if split_copy:
                    copies = [lambda lo=lo: copy(lo, lo + 2) for lo in range(0, CT, 2)]
                else:
                    copies = [lambda: copy(0, CT)]
                return ops, copies

            # ---- q/k pair projection generator (paced copy boundaries) ----
            def pair_proj_gen(t, wtiles, qTt):
                """Yields once per PE beat: 8 accumulation matmuls per group,
                then 2 empty beats around each PSUM->SBUF copy so the
                single-bank proj rotation never stalls the PE."""
                kzp = kz[t % 2]
                for wi, wt in enumerate(wtiles):
                    for sh in range(HF):
                        sl = slice(sh * 512, sh * 512 + 512)
                        ps = psum.tile(
                            [P, 512], f32, name=f"pj{t}w{wi}s{sh}",
                            tag="proj", bufs=1,
                        )
                        for ci in range(CT):
                            nc.tensor.matmul(
                                ps[:, :],
                                wt[:, ci * P : (ci + 1) * P],
                                xT(ci)[:, sl],
                                start=(ci == 0),
                                stop=(ci == CT - 1),
                            )
                            yield
                        if wi == 0:
                            nc.vector.tensor_copy(qTt[:, sl], ps[:, :])
                        else:
                            nc.vector.tensor_copy(kzp[0][0:D, sl], ps[0:D, :])
                            nc.vector.tensor_copy(kzp[1][D:P, sl], ps[D:P, :])
                        for _ in range(4):
                            yield

            def new_pair(t, eng):
                wtiles = load_wqk(t, eng)
                qTt = qkT_pool.tile([P, N], bf16, name=f"qT{t}", tag="qkT")
                return qTt, pair_proj_gen(t, wtiles, qTt)

            # ---- fused phase 1: transposes + v projection + pair-0 proj ----
            tr_ops, tr_copies = emit_tr_group(0, split_copy=True)
            for op in tr_ops:
                op()
            for cp in tr_copies:
                cp()
            # cover the tr0-copy round-trip (v(0) ci=0 waits on the first
            # xTall copy) with a couple more fillers
            for i in range(2):
                fl = psum.tile([P, 512], f32, name=f"warmb{i}", tag="proj", bufs=1)
                nc.tensor.matmul(
                    fl[:, :], ones_bf[:, 0:P], ones_bf[:, :], start=True, stop=True
                )

            qT0 = qkT_pool.tile([P, N], bf16, name="qT0", tag="qkT")
            gen0 = pair_proj_gen(0, wqk0, qT0)
            qT_cur = qT0

            # ones columns of the augmented v (emitted after the tr0 copies
            # so the DVE reaches those first)
            for sc in range(NT):
                va3 = vaug[sc].rearrange("p (h u) -> p h u", u=D + 1)
                nc.vector.memset(va3[:, :, D : D + 1], 1.0)

            # queue the remaining 7 tr groups; consume 4 ops/position during
            # sc 0-1 (all xT written by position 16); pair-0 projection beats
            # (which read wide xT column ranges) only start at sc 2.
            pend_tr = deque()
            for si in range(1, NT):
                ops, cps = emit_tr_group(si)
                pend_tr.extend(ops)
                pend_tr.extend(cps)
            owed = 0.0
            for sc in range(NT):
                v_ps = mm_tile(f"vps{sc}")
                for ci in range(CT):
                    st = dict(start=(ci == 0), stop=(ci == CT - 1))
                    for hf in range(HF):
                        sl = slice(hf * 512, hf * 512 + 512)
                        nc.tensor.matmul(
                            v_ps[:, sl],
                            xT(ci)[:, sc * P : (sc + 1) * P],
                            wv[ci][:, sl],
                            **st,
                        )
                    for _ in range(4):
                        if pend_tr:
                            pend_tr.popleft()()
                    if sc >= 2:
                        owed += 48.0 / 48.0
                        while owed >= 1.0:
                            if next(gen0, "end") == "end":
                                owed = 0.0
                                break
                            owed -= 1.0
                va3 = vaug[sc].rearrange("p (h u) -> p h u", u=D + 1)
                nc.vector.tensor_copy(
                    va3[:, :, 0:D],
                    v_ps.rearrange("p (h u) -> p h u", u=D),
                )
            for _ in gen0:
                pass

            # ---------------- phase 2: attention ----------------
            # bias chain is only needed in phase 3; emitting it here (with a
            # schedule delay) keeps it off the critical early DVE/DMA queues
            b_row = consts.tile([1, C], bf16, name="b_row", tag="b_row")
            b_f32 = consts.tile([1, C], f32, name="b_f32", tag="b_f32")
            bias_bc = consts.tile([P, C], f32, name="bias_bc", tag="bias_bc")
            with tc.tile_wait_until(0.05):
                nc.sync.dma_start(out=b_row[0:1, :], in_=bout_d[0:1, :])
                nc.vector.tensor_copy(b_f32[0:1, :], b_row[0:1, :])
                nc.gpsimd.partition_broadcast(
                    bias_bc[:, :], b_f32[0:1, :], channels=P
                )

            wos = []

            def prefetch_wo(ci):
                wo = wo_pool.tile([P, N], bf16, name=f"wo{ci}", tag="wo")
                # needed only in phase 3 — keep these DMAs from being hoisted
                # into the bandwidth-bound opening
                with tc.tile_wait_until(0.04 + 0.02 * ci):
                    nc.sync.dma_start(
                        out=wo[:, :], in_=wout_d[ci * P : (ci + 1) * P, :]
                    )
                wos.append(wo)

            av_queue = deque()  # (kind, closure): "mm" / "stage" / "b"

            def drain_av(n):
                for _ in range(n):
                    if not av_queue:
                        return
                    kind, fn = av_queue.popleft()
                    fn()
                    if kind == "stage":
                        # end the slot here: the stage copy gets a full slot
                        # of PE work before the next pass reuses the acc bank
                        return

            def enqueue_head(h, t, row0, pts):
                acc = psum.tile([P, 512], f32, name=f"acc{h}", tag="acc", bufs=1)
                oTu = oTu_pool.tile([D + 1, N], f32, name=f"oTu{h}", tag="oTu")
                rs = rs_pool.tile([1, N], f32, name=f"rs{h}", tag="rsum")

                def av(kc, hf, h=h):
                    sl = slice(hf * 512, hf * 512 + 512)
                    nc.tensor.matmul(
                        acc[0 : D + 1, :],
                        vaug[kc][:, h * (D + 1) : (h + 1) * (D + 1)],
                        pts[kc][:, sl],
                        start=(kc == 0),
                        stop=(kc == NT - 1),
                    )

                def stage(hf):
                    # the acc-bank WAR (next AV pass) waits only on the oTu
                    # copy; rs/recip/broadcast/mult chain off oTu so each
                    # 512-half of oT[t] is ready as soon as its pass is done
                    sl = slice(hf * 512, hf * 512 + 512)
                    nc.vector.tensor_copy(oTu[0 : D + 1, sl], acc[0 : D + 1, :])
                    nc.vector.tensor_copy(rs[0:1, sl], oTu[D : D + 1, sl])
                    rc = rc_pool.tile(
                        [1, 512], f32, name=f"rc{h}_{hf}", tag="recip"
                    )
                    nc.vector.reciprocal_approx_fast(rc[0:1, :], rs[0:1, sl])
                    bcs = bcs_pool.tile(
                        [D, 512], f32, name=f"bcs{h}_{hf}", tag="bcs"
                    )
                    nc.gpsimd.partition_broadcast(bcs[0:D, :], rc[0:1, :], channels=D)
                    nc.vector.tensor_mul(
                        oT[t][row0 : row0 + D, sl], oTu[0:D, sl], bcs[0:D, :]
                    )

                for kc in range(NT):
                    av_queue.append(("mm", lambda kc=kc: av(kc, 0)))
                av_queue.append(("stage", lambda: stage(0)))
                for kc in range(NT):
                    av_queue.append(("mm", lambda kc=kc: av(kc, 1)))
                av_queue.append(("stage", lambda: stage(1)))

            fill_i = [0]

            def pair7_fill():
                fl = psum.tile(
                    [P, 512], f32, name=f"fl{fill_i[0]}", tag="proj", bufs=1
                )
                fill_i[0] += 1
                nc.tensor.matmul(
                    fl[:, :], ones_bf[:, 0:P], ones_bf[:, :], start=True, stop=True
                )

            # pair 7 has no next-pair projection to interleave; use its slack
            # slots to pre-compute sc=0's out-projection (ci 0..6, bias folded
            # in) in the idle proj bank, shrinking phase 3
            o_main = [
                consts.tile([P, 512], f32, name=f"om{hf}", tag=f"om{hf}")
                for hf in range(HF)
            ]

            def pair7_work():
                # ci<=5 read heads <=11 (long done); ci=6 reads head 13's
                # normalize, which only drains out of the av queue around
                # pair-7 slot ~10 — pad with None (caller falls back to fill)
                def mmstep(ci, ps, sl):
                    return lambda: nc.tensor.matmul(
                        ps[:, :],
                        oT[ci][:, 0:P],
                        wos[ci][:, sl],
                        start=(ci == 0),
                        stop=(ci == CT - 2),
                    )

                for hf in range(HF):
                    sl = slice(hf * 512, hf * 512 + 512)
                    ps = psum.tile(
                        [P, 512], f32, name=f"oh{hf}", tag="proj", bufs=1
                    )
                    for ci in range(CT - 2):
                        yield mmstep(ci, ps, sl)
                    if hf == 0:
                        for _ in range(5):
                            yield None
                    yield mmstep(CT - 2, ps, sl)
                    nc.vector.tensor_add(o_main[hf][:, :], ps[:, :], bias_bc[:, sl])

            p7 = pair7_work()
            for t in range(HP):
                prefetch_wo(t)
                if t + 1 < HP:
                    qT_nxt, gen = new_pair(t + 1, nc.sync)
                else:
                    qT_nxt = gen = None
                owed = 0.0
                for j in range(2):
                    h = 2 * t + j
                    row0 = D * j
                    kzh = kz[t % 2][j]
                    pts = []
                    for kc in range(NT):
                        sT = mm_tile(f"s{h}_{kc}")
                        for hf in range(HF):
                            sl = slice(hf * 512, hf * 512 + 512)
                            nc.tensor.matmul(
                                sT[:, sl],
                                kzh[:, kc * P : (kc + 1) * P],
                                qT_cur[:, sl],
                                start=True,
                                stop=True,
                            )
                        pt = pT_pool.tile([P, N], bf16, name=f"pt{h}_{kc}", tag="pT")
                        nc.scalar.activation(
                            out=pt[:, :], in_=sT[:, :], func=Exp, scale=SCALE
                        )
                        pts.append(pt)
                        if kc == NT - 1:
                            enqueue_head(h, t, row0, pts)
                        drain_av(min(3, max(0, len(av_queue) - TRAIL)))
                        if gen is not None:
                            # front-load: all 48 beats done by slot 14 so the
                            # last kz copies land before the next pair's S
                            slot = j * NT + kc
                            owed += 48.0 / 14.0 if slot < 14 else 0.0
                            while owed >= 1.0:
                                if next(gen, "end") == "end":
                                    owed = 0.0
                                    break
                                owed -= 1.0
                        else:
                            step = next(p7, None)
                            if step is not None:
                                step()
                            else:
                                pair7_fill()  # covers None-padding slots too
                if gen is not None:
                    for _ in gen:
                        pass
                qT_cur = qT_nxt
            for step in p7:
                if step is not None:
                    step()

            # ---- out-proj openings interleaved with the trailing AV work ----
            o_ps = {}

            def open_steps(sc):
                o_ps[sc] = mm_tile(f"ops{sc}")
                for ci in range(CT - 1):
                    for hf in range(HF):
                        sl = slice(hf * 512, hf * 512 + 512)
                        yield lambda ci=ci, hf=hf, sl=sl, sc=sc: nc.tensor.matmul(
                            o_ps[sc][:, sl],
                            oT[ci][:, sc * P : (sc + 1) * P],
                            wos[ci][:, sl],
                            start=(ci == 0),
                            stop=False,
                        )

            def close(sc, eng):
                ci = CT - 1
                ots = []
                for hf in range(HF):
                    sl = slice(hf * 512, hf * 512 + 512)
                    nc.tensor.matmul(
                        o_ps[sc][:, sl],
                        oT[ci][:, sc * P : (sc + 1) * P],
                        wos[ci][:, sl],
                        start=False,
                        stop=True,
                    )
                for hf in range(HF):
                    sl = slice(hf * 512, hf * 512 + 512)
                    ot = io_pool.tile([P, 512], f32, name=f"ot{sc}_{hf}", tag="ot")
                    nc.vector.tensor_add(ot[:, :], o_ps[sc][:, sl], bias_bc[:, sl])
                    ots.append(ot)
                for hf in range(HF):
                    eng.dma_start(
                        out=out_d[sc * P : (sc + 1) * P, hf * 512 : hf * 512 + 512],
                        in_=ots[hf][:, :],
                    )

            def close0(eng):
                # sc=0's ci 0..6 (and bias) were pre-computed into o_main
                # during pair 7; only the ci=7 contribution remains
                ps0 = mm_tile("ops0")
                ci = CT - 1
                for hf in range(HF):
                    sl = slice(hf * 512, hf * 512 + 512)
                    nc.tensor.matmul(
                        ps0[:, sl],
                        oT[ci][:, 0:P],
                        wos[ci][:, sl],
                        start=True,
                        stop=True,
                    )
                for hf in range(HF):
                    sl = slice(hf * 512, hf * 512 + 512)
                    ot = io_pool.tile([P, 512], f32, name=f"ot0_{hf}", tag="ot")
                    nc.vector.tensor_add(ot[:, :], ps0[:, sl], o_main[hf][:, :])
                    eng.dma_start(out=out_d[0:P, sl], in_=ot[:, :])

            # interleave the trailing AV work (last ~1.5 heads) with the first
            # out-projection openings; defer each chunk's ci=7 close until
            # after the final normalize has landed.
            openers = deque()
            for sc in range(1, 4):
                openers.extend(open_steps(sc))
            while av_queue:
                drain_av(2)
                for _ in range(2):
                    if openers:
                        openers.popleft()()
            while openers:
                openers.popleft()()

            engs = [nc.sync, nc.scalar]
            seq = [("c", 1), ("c", 2), ("o", 4), ("c", 3), ("o", 5), ("c", 4),
                   ("o", 6), ("c", 5), ("o", 7), ("c", 0), ("c", 6), ("c", 7)]
            ei = 0
            for kind, sc in seq:
                if kind == "o":
                    for step in open_steps(sc):
                        step()
                elif sc == 0:
                    close0(engs[ei % 2])
                    ei += 1
                else:
                    close(sc, engs[ei % 2])
                    ei += 1

    nc.compile()
    return nc


def _get_program():
    if "nc" not in _CACHE:
        _CACHE["nc"] = _build_program()
    return _CACHE["nc"]


def _bf16(a):
    import ml_dtypes

    return np.ascontiguousarray(np.asarray(a, dtype=np.float32)).astype(
        ml_dtypes.bfloat16
    )


def _in_maps(inputs):
    x = _bf16(inputs["x"])
    w_qkv = _bf16(inputs["w_qkv"])
    w_out = _bf16(inputs["w_out"])
    b_row = _bf16(np.asarray(inputs["b_out"]).reshape(1, C))
    w_v = np.ascontiguousarray(w_qkv[:, 2 * C : 3 * C])
    wqk = np.empty((2 * HP * P, C), dtype=w_qkv.dtype)
    for t in range(HP):
        for wi, colbase in ((0, t * P), (1, C + t * P)):
            blk = w_qkv[:, colbase : colbase + P]  # [1024, 128]
            wqk[(2 * t + wi) * P : (2 * t + wi + 1) * P] = (
                blk.reshape(CT, P, P).transpose(1, 0, 2).reshape(P, C)
            )
    return [
        {"x": x[i], "w_v": w_v, "w_qk": wqk, "w_out": w_out, "b_out": b_row}
        for i in range(B)
    ]


def kernel(x, w_qkv, w_out, b_out):
    from concourse.bass_utils import run_bass_kernel_spmd

    nc = _get_program()
    in_maps = _in_maps({"x": x, "w_qkv": w_qkv, "w_out": w_out, "b_out": b_out})
    res = run_bass_kernel_spmd(nc, in_maps, core_ids=list(range(B))).results
    return np.stack([res[i]["out"] for i in range(B)], axis=0)
